# revision 6
# baseline (speedup 1.0000x reference)
"""Trainium2 Bass kernel for the CachedMPS classifier (nn_CachedMPS_68212670595935).

Matrix Product State classifier over N=784 sites, D=64 bond dim, batch 8192:
  feats = (cos(pi/2 x), sin(pi/2 x));  M0 = feats[:,0,:] @ core0
  scan 782 mid sites: M <- l2norm(M @ (c0*A0 + c1*A1));  logits = last-site contraction

Reformulation: per-step L2 norms are per-row scales that commute with the linear
step maps, so they all cancel in one final normalization (eps=1e-8 effects are
~1e-8 relative). The device scan runs un-normalized with a constant alpha folded
into the mid cores for fp32 range control (values stay in [4e-9, 1e6], verified
in numpy), and normalizes once at the end.

Device structure (per core, batch shard Bs=1024, data-parallel over 8 cores):
  state V [128, Bs] fp32r SBUF = [c0*M ; c1*M] transposed+augmented.
  per site n: P2 = Wdoub_n^T @ V      2 fp32r matmuls N=512 (Wdoub = [aA|aA], streamed from HBM)
              V' = P2 * Crep_{n+1}    2 DVE tensor_tensor ops (PSUM fp32 x fp16 -> fp32r)
  Crep (site features replicated to 128 partitions) comes from a stride-0-partition
  broadcast DMA out of a device-built fp16 feature table in DRAM - off the critical
  path entirely. Two fp16 filler matmuls per site keep the PE HAM clock-gate warm
  (cold fp32r MM = 790ns vs warm ~500ns; step is latency-bound on MM->TT->MM).
"""

import numpy as np
from contextlib import ExitStack

import concourse.bass as bass
import concourse.tile as tile
from concourse import bacc, mybir
from concourse.bass_utils import run_bass_kernel_spmd

F32 = mybir.dt.float32
F32R = mybir.dt.float32r
F16 = mybir.dt.float16
AF = mybir.ActivationFunctionType

D = 64
C = 10
N_CORES = 8
HALF_PI = float(np.pi / 2.0)
ALPHA = float(2.0 ** (-1.0 / 3.0))


def build_nc(n_sites: int, Bs: int, chunk: int = 512, fillers: int = 2):
    n_mid = n_sites - 2
    n_ftiles = (n_sites + 127) // 128
    n_chunks = Bs // chunk
    assert Bs % chunk == 0 and chunk >= 256

    nc = bacc.Bacc("TRN2", target_bir_lowering=False, debug=False)

    xT = nc.dram_tensor("xT", [n_sites, Bs], F32, kind="ExternalInput")
    Ad = nc.dram_tensor("Ad", [n_mid, 128, 128], F32R, kind="ExternalInput")
    W0 = nc.dram_tensor("W0", [2, 128], F16, kind="ExternalInput")
    WL = nc.dram_tensor("WL", [128, C], F32R, kind="ExternalInput")
    ones64 = nc.dram_tensor("ones64", [D, 1], F32R, kind="ExternalInput")
    ones10 = nc.dram_tensor("ones10", [1, C], F32R, kind="ExternalInput")
    out = nc.dram_tensor("out", [C, Bs], F32, kind="ExternalOutput")
    fdram = nc.dram_tensor("fdram", [n_sites, 2, Bs], F16)  # internal feature table

    with tile.TileContext(nc) as tc:
        with ExitStack() as ctx:
            const = ctx.enter_context(tc.tile_pool(name="const", bufs=1))
            vpool = ctx.enter_context(tc.tile_pool(name="vpool", bufs=2))
            cpool = ctx.enter_context(tc.tile_pool(name="cpool", bufs=4))
            wpool = ctx.enter_context(tc.tile_pool(name="wpool", bufs=6))
            endp = ctx.enter_context(tc.tile_pool(name="endp", bufs=1))
            pp = ctx.enter_context(tc.tile_pool(name="pp", bufs=2, space="PSUM"))
            fp = ctx.enter_context(tc.tile_pool(name="fp", bufs=1, space="PSUM"))
            cp = ctx.enter_context(tc.tile_pool(name="cp", bufs=1, space="PSUM"))

            # ---- small constants
            w0_sb = const.tile([2, 128], F16)
            nc.sync.dma_start(w0_sb[:], W0.ap())
            wl_sb = const.tile([128, C], F32R)
            nc.sync.dma_start(wl_sb[:], WL.ap())
            o64_sb = const.tile([D, 1], F32R)
            nc.sync.dma_start(o64_sb[:], ones64.ap())
            o10_sb = const.tile([1, C], F32R)
            nc.sync.dma_start(o10_sb[:], ones10.ap())

            # ---- feature build on device: fsb[p, f, t, b] then to DRAM table
            xsb = const.tile([128, n_ftiles, Bs], F32)
            if n_sites % 128 != 0:
                nc.vector.memset(xsb[:], 0.0)
            for t in range(n_ftiles):
                rows = min(128, n_sites - t * 128)
                nc.sync.dma_start(xsb[:rows, t, :], xT.ap()[t * 128:t * 128 + rows, :])
            fsb = const.tile([128, 2, n_ftiles, Bs], F16)
            b_half = const.tile([128, 1], F32)
            nc.vector.memset(b_half[:], HALF_PI)
            b_zero = const.tile([128, 1], F32)
            nc.vector.memset(b_zero[:], 0.0)
            nc.scalar.activation(fsb[:, 0], xsb[:], AF.Sin, bias=b_half[:], scale=HALF_PI)
            nc.scalar.activation(fsb[:, 1], xsb[:], AF.Sin, bias=b_zero[:], scale=HALF_PI)
            for t in range(n_ftiles):
                rows = min(128, n_sites - t * 128)
                dst = bass.AP(tensor=fdram, offset=t * 128 * 2 * Bs,
                              ap=[[2 * Bs, rows], [Bs, 2], [1, Bs]])
                nc.sync.dma_start(dst, fsb[:rows, :, t, :])

            def crep_for(s):
                ct = cpool.tile([128, Bs], F16, tag="crep")
                src = bass.AP(tensor=fdram, offset=s * 2 * Bs,
                              ap=[[Bs, 2], [0, D], [1, Bs]])
                nc.sync.dma_start(ct[:], src)
                return ct

            def mm_pair(ps, lhsT, rhs_tile):
                for c in range(n_chunks):
                    nc.tensor.matmul(ps[:, c * chunk:(c + 1) * chunk], lhsT,
                                     rhs_tile[:, c * chunk:(c + 1) * chunk],
                                     start=True, stop=True)

            def tt_pair(vt, ps, ct):
                for c in range(n_chunks):
                    nc.vector.tensor_mul(vt[:, c * chunk:(c + 1) * chunk],
                                         ps[:, c * chunk:(c + 1) * chunk],
                                         ct[:, c * chunk:(c + 1) * chunk])

            # ---- site 0 init: P2_0 = [W0|W0]^T @ feats(site0)
            st0 = const.tile([2, Bs], F16)
            nc.sync.dma_start(st0[:], bass.AP(tensor=fdram, offset=0,
                                              ap=[[Bs, 2], [1, Bs]]))
            crep = crep_for(1)
            p2 = pp.tile([128, Bs], F32, tag="p2")
            mm_pair(p2, w0_sb[:], st0[:])
            v = vpool.tile([128, Bs], F32R, tag="v")
            tt_pair(v, p2, crep)

            # ---- main scan
            for n in range(1, n_mid + 1):
                w_sb = wpool.tile([128, 128], F32R, tag="w")
                nc.sync.dma_start(w_sb[:], Ad.ap()[n - 1])
                crep = crep_for(n + 1)
                p2 = pp.tile([128, Bs], F32, tag="p2")
                mm_pair(p2, w_sb[:], v[:])
                for _ in range(fillers):
                    fill = fp.tile([128, chunk], F32, tag="fill")
                    nc.tensor.matmul(fill[:], crep[:, 0:128], crep[:, 0:chunk],
                                     start=True, stop=True, skip_group_check=True)
                v = vpool.tile([128, Bs], F32R, tag="v")
                tt_pair(v, p2, crep)
                last_p2 = p2

            # ---- endgame: logits + final normalization
            lg = pp.tile([C, Bs], F32, tag="p2")
            mm_pair(lg, wl_sb[:], v[:])
            sq = endp.tile([D, Bs], F32R)
            nc.scalar.activation(sq[:], last_p2[0:D, :], AF.Square)
            ns = cp.tile([1, Bs], F32, tag="end")
            mm_pair(ns, o64_sb[:], sq[:])
            rec = endp.tile([1, Bs], F32)
            nc.vector.reciprocal(rec[:], ns[:])
            inv = endp.tile([1, Bs], F32R)
            nc.scalar.activation(inv[:], rec[:], AF.Sqrt)
            irep = cp.tile([C, Bs], F32, tag="end")
            mm_pair(irep, o10_sb[:], inv[:])
            isb = endp.tile([C, Bs], F32)
            nc.scalar.copy(isb[:], irep[:])
            res = endp.tile([C, Bs], F32)
            nc.vector.tensor_mul(res[:], lg[:], isb[:])
            nc.sync.dma_start(out.ap(), res[:])

    nc.compile()
    return nc


def host_prep(x, core0, cores_mid, core_last, n_cores=N_CORES):
    x = np.asarray(x, np.float32)
    core0 = np.asarray(core0, np.float32)
    cores_mid = np.asarray(cores_mid, np.float32)
    core_last = np.asarray(core_last, np.float32)
    B, n_sites = x.shape
    n_mid = n_sites - 2
    Bs = B // n_cores

    Aaug = (ALPHA * cores_mid).reshape(n_mid, 2 * D, D)
    Ad = np.ascontiguousarray(np.concatenate([Aaug, Aaug], axis=2), np.float32)
    W0 = np.ascontiguousarray(
        np.concatenate([core0[:, 0, :], core0[:, 0, :]], axis=1), np.float16)
    WL = np.ascontiguousarray(core_last.reshape(2 * D, C), np.float32)
    ones64 = np.ones((D, 1), np.float32)
    ones10 = np.ones((1, C), np.float32)

    in_maps = []
    for c in range(n_cores):
        xTs = np.ascontiguousarray(x[c * Bs:(c + 1) * Bs].T)
        in_maps.append({"xT": xTs, "Ad": Ad, "W0": W0, "WL": WL,
                       "ones64": ones64, "ones10": ones10})
    return in_maps, Bs


_CACHE = {}


def _get_nc(n_sites, Bs):
    key = (n_sites, Bs)
    if key not in _CACHE:
        _CACHE[key] = build_nc(n_sites, Bs)
    return _CACHE[key]


def run(x, core0, cores_mid, core_last, trace=False, **kw):
    B, n_sites = np.asarray(x).shape
    in_maps, Bs = host_prep(x, core0, cores_mid, core_last)
    nc = _get_nc(n_sites, Bs)
    res = run_bass_kernel_spmd(nc, in_maps, core_ids=list(range(N_CORES)), trace=trace, **kw)
    logits = np.concatenate([r["out"].T for r in res.results], axis=0).astype(np.float32)
    return logits, res


def kernel(x, core0, cores_mid, core_last):
    logits, _ = run(x, core0, cores_mid, core_last)
    return logits


# revision 7
# speedup vs baseline: 1.6229x; 1.6229x over previous
"""Trainium2 Bass kernel for the CachedMPS classifier (nn_CachedMPS_68212670595935).

Matrix Product State classifier over N=784 sites, D=64 bond dim, batch 8192:
  feats = (cos(pi/2 x), sin(pi/2 x));  M0 = feats[:,0,:] @ core0
  scan 782 mid sites: M <- l2norm(M @ (c0*A0 + c1*A1));  logits = last-site contraction

Reformulation: per-step L2 norms are per-row scales that commute with the linear
step maps, so they all cancel in one final normalization (eps=1e-8 effects are
~1e-8 relative). The device scan runs un-normalized with a constant alpha folded
into the mid cores for fp32 range control (values stay within [4e-9, 1e6] for the
fixed inputs, verified in numpy), and normalizes once at the end.

Device structure (per core, batch shard Bs=1024, data-parallel over 8 cores):
  state V [128, Bs] fp32r SBUF = [c0*M ; c1*M] transposed+augmented.
  per site n:
    P2   = Wdoub_n^T @ V     2 fp32r matmuls N=512 (Wdoub = [aA|aA] streamed from HBM)
    Crep = E^T @ stage_n     2 fp16 matmuls replicating the site's (c0,c1) feature
                             rows to 128 partitions (also keeps the PE HAM-warm:
                             cold fp32r MM = 790ns vs warm ~500ns)
    CrepSB <- Crep           ACT copy PSUM->SBUF (fp16)
    V' = P2 * CrepSB         2 DVE tensor_tensor (PSUM fp32 x fp16 -> fp32r)
  The step is latency-bound on the MM -> TT -> MM serial chain (~1.4us); features
  are precomputed on device (ACT Sin) and staged per-site by a tiny 4KB DMA.
"""

import numpy as np
from contextlib import ExitStack

import concourse.bass as bass
import concourse.tile as tile
from concourse import bacc, mybir
from concourse.bass_utils import run_bass_kernel_spmd

F32 = mybir.dt.float32
F32R = mybir.dt.float32r
F16 = mybir.dt.float16
AF = mybir.ActivationFunctionType

D = 64
C = 10
N_CORES = 8
HALF_PI = float(np.pi / 2.0)
ALPHA = float(2.0 ** (-1.0 / 3.0))


def build_nc(n_sites: int, Bs: int, chunk: int = 512):
    n_mid = n_sites - 2
    n_ftiles = (n_sites + 127) // 128
    n_chunks = Bs // chunk
    assert Bs % chunk == 0 and chunk >= 256

    nc = bacc.Bacc("TRN2", target_bir_lowering=False, debug=False)

    xT = nc.dram_tensor("xT", [n_sites, Bs], F32, kind="ExternalInput")
    Ad = nc.dram_tensor("Ad", [n_mid, 128, 128], F32R, kind="ExternalInput")
    W0 = nc.dram_tensor("W0", [2, 128], F16, kind="ExternalInput")
    WL = nc.dram_tensor("WL", [128, C], F32R, kind="ExternalInput")
    Esel = nc.dram_tensor("Esel", [2, 128], F16, kind="ExternalInput")
    ones64 = nc.dram_tensor("ones64", [D, 1], F32R, kind="ExternalInput")
    ones10 = nc.dram_tensor("ones10", [1, C], F32R, kind="ExternalInput")
    out = nc.dram_tensor("out", [C, Bs], F32, kind="ExternalOutput")

    with tile.TileContext(nc) as tc:
        with ExitStack() as ctx:
            const = ctx.enter_context(tc.tile_pool(name="const", bufs=1))
            vpool = ctx.enter_context(tc.tile_pool(name="vpool", bufs=2))
            crep_sb = ctx.enter_context(tc.tile_pool(name="crep_sb", bufs=2))
            wpool = ctx.enter_context(tc.tile_pool(name="wpool", bufs=6))
            stpool = ctx.enter_context(tc.tile_pool(name="stpool", bufs=4))
            endp = ctx.enter_context(tc.tile_pool(name="endp", bufs=1))
            pp = ctx.enter_context(tc.tile_pool(name="pp", bufs=2, space="PSUM"))
            cp = ctx.enter_context(tc.tile_pool(name="cp", bufs=2, space="PSUM"))

            # ---- constants
            e_sb = const.tile([2, 128], F16)
            nc.sync.dma_start(e_sb[:], Esel.ap())
            w0_sb = const.tile([2, 128], F16)
            nc.sync.dma_start(w0_sb[:], W0.ap())
            wl_sb = const.tile([128, C], F32R)
            nc.sync.dma_start(wl_sb[:], WL.ap())
            o64_sb = const.tile([D, 1], F32R)
            nc.sync.dma_start(o64_sb[:], ones64.ap())
            o10_sb = const.tile([1, C], F32R)
            nc.sync.dma_start(o10_sb[:], ones10.ap())

            # ---- feature build: fsb[p, f, t, b] = cos/sin(pi/2 * x[t*128+p, b])
            xsb = const.tile([128, n_ftiles, Bs], F32)
            if n_sites % 128 != 0:
                nc.vector.memset(xsb[:], 0.0)
            for t in range(n_ftiles):
                rows = min(128, n_sites - t * 128)
                nc.sync.dma_start(xsb[:rows, t, :], xT.ap()[t * 128:t * 128 + rows, :])
            fsb = const.tile([128, 2, n_ftiles, Bs], F16)
            b_half = const.tile([128, 1], F32)
            nc.vector.memset(b_half[:], HALF_PI)
            b_zero = const.tile([128, 1], F32)
            nc.vector.memset(b_zero[:], 0.0)
            nc.scalar.activation(fsb[:, 0], xsb[:], AF.Sin, bias=b_half[:], scale=HALF_PI)
            nc.scalar.activation(fsb[:, 1], xsb[:], AF.Sin, bias=b_zero[:], scale=HALF_PI)

            def stage_site(s):
                st = stpool.tile([2, Bs], F16, tag="stage")
                p, t = s % 128, s // 128
                nc.sync.dma_start(st[:], fsb[p:p + 1, :, t, :])
                return st

            def mm_pair(ps, lhsT, rhs_tile):
                for c in range(n_chunks):
                    nc.tensor.matmul(ps[:, c * chunk:(c + 1) * chunk], lhsT,
                                     rhs_tile[:, c * chunk:(c + 1) * chunk],
                                     start=True, stop=True)

            def tt_pair(vt, ps, ct):
                for c in range(n_chunks):
                    nc.vector.tensor_mul(vt[:, c * chunk:(c + 1) * chunk],
                                         ps[:, c * chunk:(c + 1) * chunk],
                                         ct[:, c * chunk:(c + 1) * chunk])

            def crep_for(s):
                st = stage_site(s)
                cr = cp.tile([128, Bs], F32, tag="crep")
                mm_pair(cr, e_sb[:], st[:])
                csb = crep_sb.tile([128, Bs], F16, tag="csb")
                nc.scalar.copy(csb[:], cr[:])
                return csb

            # ---- site 0 init: P2_0 = [W0|W0]^T @ feats(site0)
            st0 = stage_site(0)
            p2 = pp.tile([128, Bs], F32, tag="p2")
            mm_pair(p2, w0_sb[:], st0[:])
            csb = crep_for(1)
            v = vpool.tile([128, Bs], F32R, tag="v")
            tt_pair(v, p2, csb)

            # ---- main scan
            for n in range(1, n_mid + 1):
                w_sb = wpool.tile([128, 128], F32R, tag="w")
                nc.sync.dma_start(w_sb[:], Ad.ap()[n - 1])
                p2 = pp.tile([128, Bs], F32, tag="p2")
                mm_pair(p2, w_sb[:], v[:])
                csb = crep_for(n + 1)
                v = vpool.tile([128, Bs], F32R, tag="v")
                tt_pair(v, p2, csb)
                last_p2 = p2

            # ---- endgame: logits + final normalization
            lg = pp.tile([C, Bs], F32, tag="p2")
            mm_pair(lg, wl_sb[:], v[:])
            sq = endp.tile([D, Bs], F32R)
            nc.scalar.activation(sq[:], last_p2[0:D, :], AF.Square)
            ns = cp.tile([1, Bs], F32, tag="crep")
            mm_pair(ns, o64_sb[:], sq[:])
            rec = endp.tile([1, Bs], F32)
            nc.vector.reciprocal(rec[:], ns[:])
            inv = endp.tile([1, Bs], F32R)
            nc.scalar.activation(inv[:], rec[:], AF.Sqrt)
            irep = cp.tile([C, Bs], F32, tag="crep")
            mm_pair(irep, o10_sb[:], inv[:])
            isb = endp.tile([C, Bs], F32)
            nc.scalar.copy(isb[:], irep[:])
            res = endp.tile([C, Bs], F32)
            nc.vector.tensor_mul(res[:], lg[:], isb[:])
            nc.sync.dma_start(out.ap(), res[:])

    nc.compile()
    return nc


def host_prep(x, core0, cores_mid, core_last, n_cores=N_CORES):
    x = np.asarray(x, np.float32)
    core0 = np.asarray(core0, np.float32)
    cores_mid = np.asarray(cores_mid, np.float32)
    core_last = np.asarray(core_last, np.float32)
    B, n_sites = x.shape
    n_mid = n_sites - 2
    Bs = B // n_cores

    Aaug = (ALPHA * cores_mid).reshape(n_mid, 2 * D, D)
    Ad = np.ascontiguousarray(np.concatenate([Aaug, Aaug], axis=2), np.float32)
    W0 = np.ascontiguousarray(
        np.concatenate([core0[:, 0, :], core0[:, 0, :]], axis=1), np.float16)
    WL = np.ascontiguousarray(core_last.reshape(2 * D, C), np.float32)
    E = np.zeros((2, 128), np.float16)
    E[0, :D] = 1.0
    E[1, D:] = 1.0
    ones64 = np.ones((D, 1), np.float32)
    ones10 = np.ones((1, C), np.float32)

    in_maps = []
    for c in range(n_cores):
        xTs = np.ascontiguousarray(x[c * Bs:(c + 1) * Bs].T)
        in_maps.append({"xT": xTs, "Ad": Ad, "W0": W0, "WL": WL, "Esel": E,
                       "ones64": ones64, "ones10": ones10})
    return in_maps, Bs


_CACHE = {}


def _get_nc(n_sites, Bs):
    key = (n_sites, Bs)
    if key not in _CACHE:
        _CACHE[key] = build_nc(n_sites, Bs)
    return _CACHE[key]


def run(x, core0, cores_mid, core_last, trace=False, **kw):
    B, n_sites = np.asarray(x).shape
    in_maps, Bs = host_prep(x, core0, cores_mid, core_last)
    nc = _get_nc(n_sites, Bs)
    res = run_bass_kernel_spmd(nc, in_maps, core_ids=list(range(N_CORES)), trace=trace, **kw)
    logits = np.concatenate([r["out"].T for r in res.results], axis=0).astype(np.float32)
    return logits, res


def kernel(x, core0, cores_mid, core_last):
    logits, _ = run(x, core0, cores_mid, core_last)
    return logits
